# revision 1
# baseline (speedup 1.0000x reference)
"""GAT-style edge-affinity layer (nn_Decode_Cora) on 8 Trainium2 NeuronCores.

Sharding: each core owns a 512-node slice of the graph (source nodes j). It
projects its own nodes (g = vert @ W), computes attention-numerator/denominator
partial sums over its 512 source nodes j for ALL 4096 destinations i, and a
ReduceScatter (in destination-slice-major layout) hands each core its 512
output destinations for the final transpose + divide + ELU.

Math: softmax rows are invariant to per-row scaling, so
    p[i,j] = mask * exp(lrelu(sl_i + sr_j))
           ∝ mask * max(exp(0.8*sl_i + sr_j), exp(0.2*sr_j))
which needs one ACT exp (per-partition bias sr_j) and one fused
scalar_tensor_tensor (max with exp(0.2*sr_j), then multiply by mask).

Phase-3 matmul orientation: stationary lhsT = per-(head,chunk) [128 j, 9]
tile ([g_h | ones]); rhs streams pm [128 j, 512 i] slices into 8 PSUM banks
(one per 512-destination slice; per-head accumulation groups live at
partition offset 9h of each bank). This replaces the baseline's 1024
weight-load-bound [128x128] matmuls with 256 stream-efficient ones.
"""

import sys

for _p in ("/opt/trn_rl_repo",):
    if _p not in sys.path:
        sys.path.append(_p)

import numpy as np
import ml_dtypes

import concourse.bass as bass
import concourse.bacc as bacc
import concourse.mybir as mybir
import concourse.tile as tile
from concourse.masks import make_identity

f32 = mybir.dt.float32
f16 = mybir.dt.float16

N = 4096          # nodes
F = 1433          # input features
FP = 1536         # padded features (12 * 128)
KT = FP // 128    # 12 contraction tiles
H = 8             # heads
DH = 8            # per-head dim
HD = H * DH       # 64
NC = 8            # cores
NL = N // NC      # 512 nodes per core
NCH = NL // 128   # 4 local j-chunks
NIS = N // 512    # 8 destination column slices
LRELU = 0.2

_STATE = {}


def _build_program(repeat=1, null=False, nocc=False, debug=False, variant='b'):
    nc = bacc.Bacc("TRN2", target_bir_lowering=False, debug=False, num_devices=NC)

    # partition-major layouts: row p holds all KT contraction tiles, so each
    # load is one DMA with multi-KB per-partition descriptors.
    vt = nc.dram_tensor("vt", [128, KT * NL], f16, kind="ExternalInput")
    vtl = nc.dram_tensor("vtl", [128, KT * NL], f16, kind="ExternalInput")
    wp = nc.dram_tensor("wp", [128, KT * HD], f16, kind="ExternalInput")
    wpl = nc.dram_tensor("wpl", [128, KT * HD], f16, kind="ExternalInput")
    # [0.8*W@a_l.T | W@a_r.T] hi/lo, partition-major: sl/sr come straight
    # from vert, with no AllGather (sl for ALL nodes is computed on every
    # core from the replicated full vert, vf).
    wa = nc.dram_tensor("wa", [128, KT * 16], f16, kind="ExternalInput")
    wal = nc.dram_tensor("wal", [128, KT * 16], f16, kind="ExternalInput")
    vf = nc.dram_tensor("vf", [128, KT * N], f16, kind="ExternalInput")
    mskt = nc.dram_tensor("mskt", [NL, N], f16, kind="ExternalInput")
    out = nc.dram_tensor("out", [NL, HD], f32, kind="ExternalOutput")

    sl2 = nc.dram_tensor("sl2", [H, N], f16)  # exp(0.8*sl), all nodes
    GR = (27, 27, 18)  # numt rows per head-group (3+3+2 heads)
    numt_g = [nc.dram_tensor(f"numt_g{g}", [NC * GR[g], 512], f32)
              for g in range(3)]
    numt_rs = [nc.dram_tensor(f"numt_rs{g}", [GR[g], 512], f32)
               for g in range(3)]

    if null:
        with tile.TileContext(nc) as tc:
            with tc.tile_pool(name="np0", bufs=1) as p0:
                t0 = p0.tile([128, 64], f16)
                t1 = p0.tile([128, 64], f32)
                for b in range(NL // 128):
                    nc.sync.dma_start(t0[:], vt[128 * b:128 * (b + 1), 0:64])
                    nc.vector.tensor_copy(t1[:], t0[:])
                    nc.sync.dma_start(out[128 * b:128 * (b + 1), :], t1[:])
        nc.compile()
        return nc

    with tile.TileContext(nc) as tc:
        with (
            tc.tile_pool(name="const", bufs=1) as cp,
            tc.tile_pool(name="psum", bufs=8, space="PSUM") as pp,
        ):
            # ---- constants / big resident tiles ----
            w_sb = cp.tile([128, KT, HD], f16)
            nc.sync.dma_start(w_sb[:], wp[:].rearrange("p (k d) -> p k d", k=KT))
            wl_sb = cp.tile([128, KT, HD], f16)
            nc.sync.dma_start(wl_sb[:], wpl[:].rearrange("p (k d) -> p k d", k=KT))
            wa_sb = cp.tile([128, KT, 16], f16)
            nc.sync.dma_start(wa_sb[:], wa[:].rearrange("p (k d) -> p k d", k=KT))
            wal_sb = cp.tile([128, KT, 16], f16)
            nc.sync.dma_start(wal_sb[:], wal[:].rearrange("p (k d) -> p k d", k=KT))
            ident = cp.tile([128, 128], f32)
            make_identity(nc, ident[:])
            msk_sb = cp.tile([128, NCH, N], f16)
            gt_sb = cp.tile([128, NL], f32)      # g^T padded to 128 partitions
            er1_sb = cp.tile([128, NCH * H], f32)  # exp(sr) per chunk, col 8c+h
            esr_sb = cp.tile([128, NCH * H], f32)  # exp(0.2 sr)
            gr_sb = cp.tile([128, NCH, H, 9], f16)  # lhsT per chunk/head: [g_h | ones]

            # ---- phase 1a: sl for ALL nodes from replicated full vert ----
            sl2_sb = cp.tile([H, N], f16, name="sl2_sb")
            with tc.tile_pool(name="vfp", bufs=3) as vfp:
                slf = [pp.tile([128, 512], f32, tag="bank", name=f"slf{s}")
                       for s in range(NIS)]
                for k in range(KT):
                    vft = vfp.tile([128, N], f16, name="vft")
                    nc.sync.dma_start(
                        vft[:], vf[:].rearrange("p (k n) -> p k n", k=KT)[:, k, :])
                    for s in range(NIS):
                        nc.tensor.matmul(slf[s][0:16, :], wa_sb[:, k, :],
                                         vft[:, 512 * s:512 * (s + 1)],
                                         start=(k == 0), stop=(k == KT - 1),
                                         skip_group_check=True)
                for s in range(NIS):
                    nc.scalar.activation(sl2_sb[:, 512 * s:512 * (s + 1)],
                                         slf[s][0:H, :],
                                         mybir.ActivationFunctionType.Exp)
            nc.sync.dma_start(sl2[:], sl2_sb[:])

            # ---- phase 1b: local projection + sr ----
            with tc.tile_pool(name="vtp", bufs=1) as vtp:
                vt_sb = vtp.tile([128, KT, NL], f16, name="vt_sb")
                nc.sync.dma_start(vt_sb[:], vt[:].rearrange("p (k n) -> p k n", k=KT))
                vtl_sb = vtp.tile([128, KT, NL], f16, name="vtl_sb")
                nc.sync.dma_start(vtl_sb[:], vtl[:].rearrange("p (k n) -> p k n", k=KT))

                for c in range(NCH):
                    nc.sync.dma_start(
                        msk_sb[:, c, :],
                        mskt[:].rearrange("(c p) i -> c p i", p=128)[c])

                # sr (local j): [16, NL] hi/lo triple product
                slr_ps = pp.tile([128, 512], f32, tag="bank", name="slr_ps")
                _sl_ops = ([(wa_sb, vt_sb, k) for k in range(KT)] +
                           [(wal_sb, vt_sb, k) for k in range(KT)] +
                           [(wa_sb, vtl_sb, k) for k in range(KT)])
                for i, (wsrc, vsrc, k) in enumerate(_sl_ops):
                    nc.tensor.matmul(slr_ps[0:16, :], wsrc[:, k, :], vsrc[:, k, :],
                                     start=(i == 0), stop=(i == len(_sl_ops) - 1))

                # sr^T -> per-chunk per-partition scalars via PE transpose
                slr_sb = cp.tile([16, NL], f32, name="slr_sb")
                nc.vector.tensor_copy(slr_sb[:], slr_ps[0:16, :])
                for c in range(NCH):
                    srt_ps = pp.tile([128, 512], f32, tag="bank", name="srt_ps")
                    nc.tensor.transpose(srt_ps[:, 0:16],
                                        slr_sb[0:16, 128 * c:128 * (c + 1)],
                                        ident[0:16, 0:16])
                    nc.scalar.activation(er1_sb[:, H * c:H * (c + 1)],
                                         srt_ps[:, H:16],
                                         mybir.ActivationFunctionType.Exp)
                    nc.scalar.activation(esr_sb[:, H * c:H * (c + 1)],
                                         srt_ps[:, H:16],
                                         mybir.ActivationFunctionType.Exp, scale=0.2)

                # projection g^T (feeds only the gr lhsT tiles now)
                nc.vector.memset(gt_sb[64:128, :], 0.0)
                gt_ps = pp.tile([128, 512], f32, tag="bank", name="gt_ps")
                _gt_ops = ([(w_sb, vt_sb, k) for k in range(KT)] +
                           [(w_sb, vtl_sb, k) for k in range(KT)] +
                           [(wl_sb, vt_sb, k) for k in range(KT)])
                for i, (wsrc, vsrc, k) in enumerate(_gt_ops):
                    nc.tensor.matmul(gt_ps[0:HD, :], wsrc[:, k, :], vsrc[:, k, :],
                                     start=(i == 0), stop=(i == len(_gt_ops) - 1))
                nc.vector.tensor_copy(gt_sb[0:HD, :], gt_ps[0:HD, :])

                for c in range(NCH):
                    # g for this chunk via PE transpose of gt: [128 nodes, 64]
                    gtr_ps = pp.tile([128, 512], f32, tag="bank", name="gtr_ps")
                    nc.tensor.transpose(gtr_ps[:, 0:HD],
                                        gt_sb[0:HD, 128 * c:128 * (c + 1)],
                                        ident[0:HD, 0:HD])
                    # lhsT tile: [g_h | ones] interleaved, 9 cols per head
                    nc.vector.tensor_copy(
                        gr_sb[:, c, :, 0:8],
                        gtr_ps[:, 0:HD].rearrange("p (h d) -> p h d", d=8))
                    nc.vector.memset(gr_sb[:, c, :, 8], 1.0)

            # ---- phase 3: main attention loop ----
            # PE output base partition must be in {0, 32, 64}, so 3 heads
            # share each PSUM bank at those offsets; heads run in groups of 3.
            with (
                tc.tile_pool(name="slbp", bufs=3) as slbp,
                tc.tile_pool(name="tp", bufs=3) as tp,
                tc.tile_pool(name="pmp", bufs=4) as pmp,
                tc.tile_pool(name="nhp", bufs=4) as nhp,
                tc.tile_pool(name="small", bufs=4) as sp,
            ):
              for _rep in range(repeat):
                for grp in range(3):
                    heads = [3 * grp + k for k in range(3) if 3 * grp + k < H]
                    banks = [pp.tile([128, 512], f32, tag="bank",
                                     name=f"bank{grp}_{s}") for s in range(NIS)]
                    for h in heads:
                        off = 32 * (h - 3 * grp)
                        slb = slbp.tile([128, N], f16, name="slb")
                        nc.sync.dma_start(
                            slb[:], sl2[h:h + 1, :].to_broadcast([128, N]))
                        for c in range(NCH):
                            # t = exp(0.8sl_i)*exp(sr_j); ~3/4 of tiles on the
                            # Scalar engine (Copy with per-partition scale),
                            # the rest on the DVE (tensor_scalar mult) to
                            # balance engine busy time.
                            t = tp.tile([128, N], f16, name="texp")
                            if h % 4 == 3:
                                nc.vector.tensor_scalar(
                                    t[:], slb[:],
                                    er1_sb[:, H * c + h:H * c + h + 1],
                                    None, mybir.AluOpType.mult)
                            else:
                                nc.scalar.activation(
                                    t[:], slb[:],
                                    mybir.ActivationFunctionType.Copy,
                                    scale=er1_sb[:, H * c + h:H * c + h + 1])
                            u = tp.tile([128, N], f16, name="umax")
                            nc.vector.tensor_scalar(u[:], t[:],
                                                    esr_sb[:, H * c + h:H * c + h + 1],
                                                    None, mybir.AluOpType.max)
                            pm = pmp.tile([128, N], f16, name="pm")
                            nc.vector.tensor_tensor(pm[:], u[:], msk_sb[:, c, :],
                                                    mybir.AluOpType.mult)
                            for s in range(NIS):
                                nc.tensor.matmul(banks[s][off:off + 9, :],
                                                 gr_sb[:, c, h, :],
                                                 pm[:, 512 * s:512 * (s + 1)],
                                                 start=(c == 0),
                                                 stop=(c == NCH - 1),
                                                 skip_group_check=True)
                    # ---- evict this group: PSUM -> SBUF -> DRAM ----
                    nrow = 32 * (len(heads) - 1) + 9
                    rg = GR[grp]
                    for s in range(NIS):
                        nh = nhp.tile([128, 512], f32, name="nh")
                        eng = nc.scalar if s % 2 == 0 else nc.vector
                        if eng is nc.scalar:
                            nc.scalar.activation(nh[0:nrow, :], banks[s][0:nrow, :],
                                                 mybir.ActivationFunctionType.Copy)
                        else:
                            nc.vector.tensor_copy(nh[0:nrow, :], banks[s][0:nrow, :])
                        for k, h in enumerate(heads):
                            nc.sync.dma_start(
                                numt_g[grp][rg * s + 9 * (h - 3 * grp):
                                            rg * s + 9 * (h - 3 * grp) + 9, :],
                                nh[32 * k:32 * k + 9, :])
                    # ---- ReduceScatter this group (overlaps next group) ----
                    if nocc:
                        nc.sync.dma_start(numt_rs[grp][:], numt_g[grp][0:rg, :])
                    else:
                        nc.gpsimd.collective_compute(
                            "ReduceScatter", mybir.AluOpType.add,
                            replica_groups=[list(range(NC))],
                            ins=[numt_g[grp][:].opt()],
                            outs=[numt_rs[grp][:].opt()],
                        )
              if True:
                # ---- phase 6: transpose + divide + ELU ----
                nf = sp.tile([128, 512], f32, name="nf")
                nc.sync.dma_start(nf[0:27, :], numt_rs[0][:])
                nc.sync.dma_start(nf[27:54, :], numt_rs[1][:])
                nc.sync.dma_start(nf[54:72, :], numt_rs[2][:])
                for b in range(NL // 128):
                    tps = pp.tile([128, 512], f32, tag="bank", name="tps")
                    nc.tensor.transpose(tps[:, 0:72],
                                        nf[0:72, 128 * b:128 * (b + 1)],
                                        ident[0:72, 0:72])
                    tpr = tps[:, 0:72].rearrange("p (h k) -> p h k", k=9)
                    rec = sp.tile([128, H], f32, name="rec")
                    nc.vector.reciprocal(rec[:], tpr[:, :, 8])
                    aout = sp.tile([128, HD], f32, name="aout")
                    for h in range(H):
                        nc.vector.tensor_scalar(aout[:, 8 * h:8 * (h + 1)],
                                                tpr[:, h, 0:8],
                                                rec[:, h:h + 1], None,
                                                mybir.AluOpType.mult)
                    # elu(x) = relu(x) - 1 + exp(min(x, 0))
                    xm = sp.tile([128, HD], f32, name="xm")
                    nc.vector.tensor_scalar(xm[:], aout[:], 0.0, None, mybir.AluOpType.min)
                    ex = sp.tile([128, HD], f32, name="ex")
                    nc.scalar.activation(ex[:], xm[:], mybir.ActivationFunctionType.Exp)
                    r1 = sp.tile([128, HD], f32, name="r1")
                    nc.vector.tensor_scalar(r1[:], aout[:], 0.0, -1.0,
                                            mybir.AluOpType.max, mybir.AluOpType.add)
                    ot = sp.tile([128, HD], f32, name="ot")
                    nc.vector.tensor_tensor(ot[:], ex[:], r1[:], mybir.AluOpType.add)
                    nc.sync.dma_start(out[128 * b:128 * (b + 1), :], ot[:])

    nc.compile()
    return nc


def _prep_inputs(vert, edge, W, a_l, a_r):
    vert = np.asarray(vert, dtype=np.float32)
    edge = np.asarray(edge)
    W = np.asarray(W, dtype=np.float32)
    a_l = np.asarray(a_l, dtype=np.float32)
    a_r = np.asarray(a_r, dtype=np.float32)

    vtp32 = np.zeros((FP, N), dtype=np.float32)
    vtp32[:F] = vert.T
    vtp = vtp32.astype(np.float16)
    vtl = (vtp32 - vtp.astype(np.float32)).astype(np.float16)
    wp32 = np.zeros((FP, HD), dtype=np.float32)
    wp32[:F] = W
    wp = wp32.astype(np.float16)
    wpl = (wp32 - wp.astype(np.float32)).astype(np.float16)
    # partition-major: [FP, X] = [(k p), X] -> [p, (k X)]
    def _pmaj(a):
        return np.ascontiguousarray(
            a.reshape(KT, 128, -1).transpose(1, 0, 2).reshape(128, -1))
    wp_pm = _pmaj(wp)
    wpl_pm = _pmaj(wpl)

    # [0.8*W@a_l.T | W@a_r.T] -> [FP, 16] hi/lo, partition-major
    wa32 = np.zeros((FP, 16), dtype=np.float32)
    w3 = W.reshape(F, H, DH)
    wa32[:F, 0:8] = 0.8 * (w3 * a_l[None]).sum(-1)
    wa32[:F, 8:16] = (w3 * a_r[None]).sum(-1)
    wa = wa32.astype(np.float16)
    wal = (wa32 - wa.astype(np.float32)).astype(np.float16)
    wa_pm = _pmaj(wa)
    wal_pm = _pmaj(wal)

    maskT = (edge != 0).astype(np.float16)  # [i, j] -> transpose below
    vf_pm = _pmaj(vtp)

    in_maps = []
    for c in range(NC):
        sl = slice(512 * c, 512 * (c + 1))
        in_maps.append({
            "vt": _pmaj(vtp[:, sl]),
            "vtl": _pmaj(vtl[:, sl]),
            "wp": wp_pm,
            "wpl": wpl_pm,
            "wa": wa_pm,
            "wal": wal_pm,
            "vf": vf_pm,
            "mskt": np.ascontiguousarray(maskT[:, sl].T),
        })
    return in_maps


def _get_runner(repeat=1, null=False, variant='b'):
    """Build (once) and return a callable in_maps -> list of per-core outputs."""
    key = f"runner{repeat}_{null}_{variant}"
    if key in _STATE:
        return _STATE[key]

    nc = _build_program(repeat, null, variant=variant)

    import jax
    from jax.sharding import Mesh, PartitionSpec
    from jax.experimental.shard_map import shard_map
    from concourse import bass2jax
    from concourse.bass2jax import _bass_exec_p, partition_id_tensor

    bass2jax.install_neuronx_cc_hook()

    partition_name = nc.partition_id_tensor.name if nc.partition_id_tensor else None
    in_names, out_names, out_avals, zero_shapes = [], [], [], []
    for alloc in nc.m.functions[0].allocations:
        if not isinstance(alloc, mybir.MemoryLocationSet):
            continue
        name = alloc.memorylocations[0].name
        if alloc.kind == "ExternalInput":
            if name != partition_name:
                in_names.append(name)
        elif alloc.kind == "ExternalOutput":
            shape = tuple(alloc.tensor_shape)
            dtype = mybir.dt.np(alloc.dtype)
            out_names.append(name)
            out_avals.append(jax.core.ShapedArray(shape, dtype))
            zero_shapes.append((shape, dtype))
    n_params = len(in_names)
    n_outs = len(out_avals)
    all_in_names = list(in_names) + list(out_names)
    if partition_name is not None:
        all_in_names.append(partition_name)
    donate = tuple(range(n_params, n_params + n_outs))

    def _body(*args):
        operands = list(args)
        if partition_name is not None:
            operands.append(partition_id_tensor())
        outs = _bass_exec_p.bind(
            *operands,
            out_avals=tuple(out_avals),
            in_names=tuple(all_in_names),
            out_names=tuple(out_names),
            lowering_input_output_aliases=(),
            sim_require_finite=True,
            sim_require_nnan=True,
            nc=nc,
        )
        return tuple(outs)

    devices = jax.devices()[:NC]
    mesh = Mesh(np.asarray(devices), ("core",))
    in_specs = (PartitionSpec("core"),) * (n_params + n_outs)
    out_specs = (PartitionSpec("core"),) * n_outs
    sharded = jax.jit(
        shard_map(_body, mesh=mesh, in_specs=in_specs, out_specs=out_specs,
                  check_rep=False),
        donate_argnums=donate, keep_unused=True,
    )

    def runner(in_maps):
        concat_in = [
            np.concatenate([np.asarray(in_maps[c][nm]) for c in range(NC)], axis=0)
            for nm in in_names
        ]
        concat_zeros = [
            np.zeros((NC * s[0], *s[1:]), dt) for (s, dt) in zero_shapes
        ]
        out_arrs = sharded(*concat_in, *concat_zeros)
        out_arrs = [np.asarray(a) for a in out_arrs]
        return [
            {nm: out_arrs[i].reshape(NC, *out_avals[i].shape)[c]
             for i, nm in enumerate(out_names)}
            for c in range(NC)
        ]

    _STATE[key] = runner
    _STATE[f"internals{repeat}_{null}_{variant}"] = {
        "sharded": sharded, "in_names": in_names, "zero_shapes": zero_shapes,
        "mesh": mesh, "out_names": out_names, "out_avals": out_avals,
    }
    return runner


def kernel(vert, edge, W, a_l, a_r):
    in_maps = _prep_inputs(vert, edge, W, a_l, a_r)
    runner = _get_runner()
    results = runner(in_maps)
    return np.concatenate([results[c]["out"] for c in range(NC)], axis=0)



# revision 9
# speedup vs baseline: 236.6366x; 236.6366x over previous
"""GAT-style edge-affinity layer (nn_Decode_Cora) on 8 Trainium2 NeuronCores.

Sharding: each core owns a 512-node slice of source nodes j. It projects its
own nodes (g, sl, sr from vert @ [Wa_l | Wa_r | W] in one PSUM pass), AllGathers
the 8KB exp(0.8*sl) vector so every core knows all destinations' sl, computes
attention-numerator/denominator partial sums over its 512 j for ALL 4096
destinations i, and a ReduceScatter (destination-slice-major) hands each core
its 512 output destinations for the final transpose + divide + ELU.

Math: softmax rows are invariant to per-row scaling, so with x = sl_i + sr_j
    p[i,j] = mask * exp(lrelu(x)) / exp(0.2*sl_i)
           = mask * e02_j * (1 + relu(ESL_i * r_j - 1))
where ESL_i = exp(0.8*sl_i), r_j = exp(0.8*sr_j), e02_j = exp(0.2*sr_j).
Per [128 j, 4096 i] tile that is ONE ScalarE op (Relu with per-partition
scale r_j, bias -1) and ONE fused DVE scalar_tensor_tensor ((v+1)*mask);
e02_j is folded into the matmul lhsT ([g_h | 1] * e02).

Head groups (4, 3, 1) at PSUM partition offsets {0,32,64,96}; each group's
partials ReduceScatter (f16) overlaps the next group's compute, so only the
last 1-head group's tiny RS (74KB) sits in the tail.
"""

import sys

for _p in ("/opt/trn_rl_repo",):
    if _p not in sys.path:
        sys.path.append(_p)

import numpy as np
import ml_dtypes

import concourse.bass as bass
import concourse.bacc as bacc
import concourse.mybir as mybir
import concourse.tile as tile
from concourse.masks import make_identity

f32 = mybir.dt.float32
f16 = mybir.dt.float16

N = 4096          # nodes
F = 1433          # input features
FP = 1536         # padded features (12 * 128)
KT = FP // 128    # 12 contraction tiles
H = 8             # heads
DH = 8            # per-head dim
HD = H * DH       # 64
NC = 8            # cores
NL = N // NC      # 512 nodes per core
NCH = NL // 128   # 4 local j-chunks
NIS = N // 512    # 8 destination column slices
GROUPS = [[0, 1, 2, 3], [4, 5, 6], [7]]
GR = [9 * len(g) for g in GROUPS]

_STATE = {}


def _build_program(repeat=1, null=False, nocc=False, debug=False, variant='b'):
    nc = bacc.Bacc("TRN2", target_bir_lowering=False, debug=False, num_devices=NC)

    # partition-major layouts: row p holds all KT contraction tiles.
    vt = nc.dram_tensor("vt", [128, KT * NL], f16, kind="ExternalInput")
    # [0.8*W@a_l.T | W@a_r.T | W], partition-major
    wq = nc.dram_tensor("wq", [128, KT * 80], f16, kind="ExternalInput")
    mskt = nc.dram_tensor("mskt", [NL, N], f16, kind="ExternalInput")
    out = nc.dram_tensor("out", [NL, HD], f32, kind="ExternalOutput")

    el_loc = nc.dram_tensor("el_loc", [H, NL], f16)     # exp(0.8*sl) local j
    sl_all = nc.dram_tensor("sl_all", [NC * H, NL], f16)  # AllGather output
    numt_g = [nc.dram_tensor(f"numt_g{g}", [NC * GR[g], 512], f16)
              for g in range(3)]
    numt_rs = [nc.dram_tensor(f"numt_rs{g}", [GR[g], 512], f16)
               for g in range(3)]

    if null:
        with tile.TileContext(nc) as tc:
            with tc.tile_pool(name="np0", bufs=1) as p0:
                t0 = p0.tile([128, 64], f16)
                t1 = p0.tile([128, 64], f32)
                for b in range(NL // 128):
                    nc.sync.dma_start(t0[:], vt[:, 64 * b:64 * (b + 1)])
                    nc.vector.tensor_copy(t1[:], t0[:])
                    nc.sync.dma_start(out[128 * b:128 * (b + 1), :], t1[:])
        nc.compile()
        return nc

    AF = mybir.ActivationFunctionType
    OP = mybir.AluOpType

    with tile.TileContext(nc) as tc:
        with (
            tc.tile_pool(name="const", bufs=1) as cp,
            tc.tile_pool(name="psum", bufs=8, space="PSUM") as pp,
        ):
            # ---- resident tiles / constants ----
            wq_sb = cp.tile([128, KT, 80], f16)
            nc.sync.dma_start(wq_sb[:], wq[:].rearrange("p (k d) -> p k d", k=KT))
            vt_sb = cp.tile([128, KT, NL], f16)
            nc.sync.dma_start(vt_sb[:], vt[:].rearrange("p (k n) -> p k n", k=KT))
            msk_sb = cp.tile([128, NCH, N], f16)
            nc.sync.dma_start(msk_sb[:],
                              mskt[:].rearrange("(c p) i -> p c i", p=128))
            ident = cp.tile([128, 128], f16)
            make_identity(nc, ident[:])
            neg1 = cp.tile([128, 1], f32)
            nc.vector.memset(neg1[:], -1.0)
            r_sb = cp.tile([128, NCH * H], f32)   # exp(0.8*sr), col 8c+h
            e2_sb = cp.tile([128, NCH * H], f32)  # exp(0.2*sr)
            g2_sb = cp.tile([128, NCH, H, 9], f16)  # lhsT: e02 * [g_h | 1]
            el_sb = cp.tile([8, NL], f16)
            nf = cp.tile([72, 512], f16)          # gathered numerators, row 9h+q

            # ---- phase 1a: local sl (head-major) -> AllGather ----
            slr_ps = pp.tile([128, 512], f32, tag="bank", name="slr")
            for k in range(KT):
                nc.tensor.matmul(slr_ps[0:8, :], wq_sb[:, k, 0:8],
                                 vt_sb[:, k, :],
                                 start=(k == 0), stop=(k == KT - 1))
            nc.scalar.activation(el_sb[:], slr_ps[0:8, :], AF.Exp)
            nc.sync.dma_start(el_loc[:], el_sb[:])
            if nocc:
                for r in range(NC):
                    nc.sync.dma_start(sl_all[8 * r:8 * (r + 1), :], el_loc[:])
            else:
                nc.gpsimd.collective_compute(
                    "AllGather", OP.bypass,
                    replica_groups=[list(range(NC))],
                    ins=[el_loc[:].opt()],
                    outs=[sl_all[:].opt()],
                )

            # ---- phase 1b: node-major projection: sr scalars + lhsT tiles ----
            for c in range(NCH):
                pch = pp.tile([128, 512], f32, tag="bank", name=f"pch{c}")
                for k in range(KT):
                    nc.tensor.matmul(pch[:, 0:72],
                                     vt_sb[:, k, 128 * c:128 * (c + 1)],
                                     wq_sb[:, k, 8:80],
                                     start=(k == 0), stop=(k == KT - 1))
                nc.scalar.activation(r_sb[:, 8 * c:8 * (c + 1)], pch[:, 0:8],
                                     AF.Exp, scale=0.8)
                nc.scalar.activation(e2_sb[:, 8 * c:8 * (c + 1)], pch[:, 0:8],
                                     AF.Exp, scale=0.2)
                for h in range(H):
                    nc.vector.tensor_scalar(
                        g2_sb[:, c, h, 0:8], pch[:, 8 + 8 * h:16 + 8 * h],
                        e2_sb[:, 8 * c + h:8 * c + h + 1], None, OP.mult)
                    nc.vector.tensor_copy(g2_sb[:, c, h, 8:9],
                                          e2_sb[:, 8 * c + h:8 * c + h + 1])

            # ---- phase 3: main attention loop ----
            with (
                tc.tile_pool(name="slbp", bufs=1) as slbp,
                tc.tile_pool(name="tp", bufs=4) as tp,
                tc.tile_pool(name="pmp", bufs=4) as pmp,
                tc.tile_pool(name="nhp", bufs=4) as nhp,
                tc.tile_pool(name="sp", bufs=4) as sp,
            ):
                slb = []
                for h in range(H):
                    t = slbp.tile([128, N], f16, name=f"slb{h}")
                    nc.sync.dma_start(
                        t[:].rearrange("p (c n) -> p c n", c=NC),
                        sl_all[:].rearrange("(c h) n -> h c n", h=H)[h:h + 1]
                        .to_broadcast([128, NC, NL]))
                    slb.append(t)

                def emit_evict(gi, heads, banks):
                    nrow = 32 * (len(heads) - 1) + 9
                    rg = GR[gi]
                    for s in range(NIS):
                        nh = nhp.tile([128, 512], f16, name="nh")
                        if s % 2 == 0:
                            nc.scalar.activation(nh[0:nrow, :],
                                                 banks[s][0:nrow, :], AF.Copy)
                        else:
                            nc.vector.tensor_copy(nh[0:nrow, :],
                                                  banks[s][0:nrow, :])
                        nc.sync.dma_start(
                            numt_g[gi][rg * s:rg * (s + 1), :]
                            .rearrange("(k r) i -> k r i", r=9),
                            nh[:].rearrange("(k r) i -> k r i", r=32)
                            [0:len(heads), 0:9, :])
                    if nocc:
                        nc.sync.dma_start(numt_rs[gi][:], numt_g[gi][0:rg, :])
                    else:
                        nc.gpsimd.collective_compute(
                            "ReduceScatter", OP.add,
                            replica_groups=[list(range(NC))],
                            ins=[numt_g[gi][:].opt()],
                            outs=[numt_rs[gi][:].opt()],
                        )
                    nc.sync.dma_start(
                        nf[9 * heads[0]:9 * heads[0] + rg, :], numt_rs[gi][:])

                pending = None
                for gi, heads in enumerate(GROUPS):
                    banks = [pp.tile([128, 512], f32, tag="bank",
                                     name=f"bk{gi}_{s}") for s in range(NIS)]
                    for hi, h in enumerate(heads):
                        off = 32 * hi
                        for c in range(NCH):
                            v = tp.tile([128, N], f16, name="v")
                            nc.scalar.activation(
                                v[:], slb[h][:], AF.Relu, bias=neg1[:],
                                scale=r_sb[:, 8 * c + h:8 * c + h + 1])
                            pm = pmp.tile([128, N], f16, name="pm")
                            nc.vector.scalar_tensor_tensor(
                                pm[:], v[:], 1.0, msk_sb[:, c, :],
                                OP.add, OP.mult)
                            for s in range(NIS):
                                nc.tensor.matmul(banks[s][off:off + 9, :],
                                                 g2_sb[:, c, h, :],
                                                 pm[:, 512 * s:512 * (s + 1)],
                                                 start=(c == 0),
                                                 stop=(c == NCH - 1),
                                                 tile_position=(0, off),
                                                 skip_group_check=True)
                        if hi == 0 and pending is not None:
                            emit_evict(*pending)
                            pending = None
                    pending = (gi, heads, banks)
                emit_evict(*pending)

                # ---- phase 6: transpose + divide + ELU ----
                for b in range(NL // 128):
                    tps = pp.tile([128, 1024], f16, tag="bank", name=f"tps{b}")
                    nc.tensor.transpose(tps[:, 0:72],
                                        nf[0:72, 128 * b:128 * (b + 1)],
                                        ident[0:72, 0:72])
                    tpr = tps[:, 0:72].rearrange("p (h k) -> p h k", k=9)
                    rec = sp.tile([128, H], f32, name="rec")
                    nc.vector.reciprocal(rec[:], tpr[:, :, 8])
                    aout = sp.tile([128, HD], f32, name="aout")
                    for h in range(H):
                        nc.vector.tensor_scalar(aout[:, 8 * h:8 * (h + 1)],
                                                tpr[:, h, 0:8],
                                                rec[:, h:h + 1], None,
                                                OP.mult)
                    # elu(x) = relu(x) - 1 + exp(min(x, 0))
                    xm = sp.tile([128, HD], f32, name="xm")
                    nc.vector.tensor_scalar(xm[:], aout[:], 0.0, None, OP.min)
                    ex = sp.tile([128, HD], f32, name="ex")
                    nc.scalar.activation(ex[:], xm[:], AF.Exp)
                    r1 = sp.tile([128, HD], f32, name="r1")
                    nc.vector.tensor_scalar(r1[:], aout[:], 0.0, -1.0,
                                            OP.max, OP.add)
                    ot = sp.tile([128, HD], f32, name="ot")
                    nc.vector.tensor_tensor(ot[:], ex[:], r1[:], OP.add)
                    nc.sync.dma_start(out[128 * b:128 * (b + 1), :], ot[:])

    nc.compile()
    return nc


def _prep_inputs(vert, edge, W, a_l, a_r):
    vert = np.asarray(vert, dtype=np.float32)
    edge = np.asarray(edge)
    W = np.asarray(W, dtype=np.float32)
    a_l = np.asarray(a_l, dtype=np.float32)
    a_r = np.asarray(a_r, dtype=np.float32)

    vtp32 = np.zeros((FP, N), dtype=np.float32)
    vtp32[:F] = vert.T
    vtp = vtp32.astype(np.float16)

    # [0.8*W@a_l.T | W@a_r.T | W] -> [FP, 80], partition-major
    wq32 = np.zeros((FP, 80), dtype=np.float32)
    w3 = W.reshape(F, H, DH)
    wq32[:F, 0:8] = 0.8 * (w3 * a_l[None]).sum(-1)
    wq32[:F, 8:16] = (w3 * a_r[None]).sum(-1)
    wq32[:F, 16:80] = W
    wq = wq32.astype(np.float16)

    # partition-major: [FP, X] = [(k p), X] -> [p, (k X)]
    def _pmaj(a):
        return np.ascontiguousarray(
            a.reshape(KT, 128, -1).transpose(1, 0, 2).reshape(128, -1))

    wq_pm = _pmaj(wq)
    maskT = (edge != 0).astype(np.float16)  # [i, j]

    in_maps = []
    for c in range(NC):
        sl = slice(512 * c, 512 * (c + 1))
        in_maps.append({
            "vt": _pmaj(vtp[:, sl]),
            "wq": wq_pm,
            "mskt": np.ascontiguousarray(maskT[:, sl].T),
        })
    return in_maps


def _get_runner(repeat=1, null=False, variant='b'):
    """Build (once) and return a callable in_maps -> list of per-core outputs."""
    key = f"runner{repeat}_{null}_{variant}"
    if key in _STATE:
        return _STATE[key]

    nc = _build_program(repeat, null, variant=variant)

    import jax
    from jax.sharding import Mesh, PartitionSpec
    from jax.experimental.shard_map import shard_map
    from concourse import bass2jax
    from concourse.bass2jax import _bass_exec_p, partition_id_tensor

    bass2jax.install_neuronx_cc_hook()

    partition_name = nc.partition_id_tensor.name if nc.partition_id_tensor else None
    in_names, out_names, out_avals, zero_shapes = [], [], [], []
    for alloc in nc.m.functions[0].allocations:
        if not isinstance(alloc, mybir.MemoryLocationSet):
            continue
        name = alloc.memorylocations[0].name
        if alloc.kind == "ExternalInput":
            if name != partition_name:
                in_names.append(name)
        elif alloc.kind == "ExternalOutput":
            shape = tuple(alloc.tensor_shape)
            dtype = mybir.dt.np(alloc.dtype)
            out_names.append(name)
            out_avals.append(jax.core.ShapedArray(shape, dtype))
            zero_shapes.append((shape, dtype))
    n_params = len(in_names)
    n_outs = len(out_avals)
    all_in_names = list(in_names) + list(out_names)
    if partition_name is not None:
        all_in_names.append(partition_name)
    donate = tuple(range(n_params, n_params + n_outs))

    def _body(*args):
        operands = list(args)
        if partition_name is not None:
            operands.append(partition_id_tensor())
        outs = _bass_exec_p.bind(
            *operands,
            out_avals=tuple(out_avals),
            in_names=tuple(all_in_names),
            out_names=tuple(out_names),
            lowering_input_output_aliases=(),
            sim_require_finite=True,
            sim_require_nnan=True,
            nc=nc,
        )
        return tuple(outs)

    devices = jax.devices()[:NC]
    mesh = Mesh(np.asarray(devices), ("core",))
    in_specs = (PartitionSpec("core"),) * (n_params + n_outs)
    out_specs = (PartitionSpec("core"),) * n_outs
    sharded = jax.jit(
        shard_map(_body, mesh=mesh, in_specs=in_specs, out_specs=out_specs,
                  check_rep=False),
        donate_argnums=donate, keep_unused=True,
    )

    def runner(in_maps):
        concat_in = [
            np.concatenate([np.asarray(in_maps[c][nm]) for c in range(NC)], axis=0)
            for nm in in_names
        ]
        concat_zeros = [
            np.zeros((NC * s[0], *s[1:]), dt) for (s, dt) in zero_shapes
        ]
        out_arrs = sharded(*concat_in, *concat_zeros)
        out_arrs = [np.asarray(a) for a in out_arrs]
        return [
            {nm: out_arrs[i].reshape(NC, *out_avals[i].shape)[c]
             for i, nm in enumerate(out_names)}
            for c in range(NC)
        ]

    _STATE[key] = runner
    _STATE[f"internals{repeat}_{null}_{variant}"] = {
        "sharded": sharded, "in_names": in_names, "zero_shapes": zero_shapes,
        "mesh": mesh, "out_names": out_names, "out_avals": out_avals,
    }
    return runner


def kernel(vert, edge, W, a_l, a_r):
    in_maps = _prep_inputs(vert, edge, W, a_l, a_r)
    runner = _get_runner()
    results = runner(in_maps)
    return np.concatenate([results[c]["out"] for c in range(NC)], axis=0)
